# revision 1
# baseline (speedup 1.0000x reference)
"""Bass/Tile kernel builder for the XCA-style attention block.

Per-core program (one batch): x [C, HW] bf16 -> y [C, HW] f32.

Pipeline:
  kv0 = Wkv @ x + kv_b                 (PE GEMM, channels-on-partitions)
  k, v = dwconv3x3(kv0) + dw_b         (DVE scalar_tensor_tensor chains and/or
                                        PE diagonal-matmul PSUM accumulation)
  norms2[d] = sum_n k[d,n]^2           (ACT Square accum_out)
  kT = transpose(k)                    (DMA xbar)
  S = qT @ kT (full gram, 3 M-chunks)  (PE, PSUM accumulated per block,
                                        added into SBUF S_acc)
  rnorm = 1/max(sqrt(norms2),eps)      (ACT sqrt + DVE reciprocal + Newton)
  logits/softmax per head              (small DVE/ACT ops)
  M_b^T = per-head attn @ projT        (PE small matmuls)
  y = M_b @ v + proj_b                 (PE GEMM over v re-streamed from DRAM)
"""
import contextlib
from contextlib import ExitStack

import numpy as np
import ml_dtypes

import concourse.bass as bass
import concourse.mybir as mybir
import concourse.tile as tile
from concourse import bacc

bf16 = mybir.dt.bfloat16
f32 = mybir.dt.float32
AF = mybir.ActivationFunctionType
ALU = mybir.AluOpType
AX = mybir.AxisListType

C = 384
C2 = 768
HEADS = 8
HD = 48
CC = 3            # 128-chunks for C
OC = 6            # 128-chunks for 2C
PS = 512          # psum chunk (one f32 bank)


def head_pieces():
    """Split each head's 48-channel range at 128-partition boundaries.

    Per head: list of (mc, p0, p1, s0): global channels
    [mc*128+p0, mc*128+p1) == within-head channels [s0, s0+(p1-p0)).
    """
    out = []
    for h in range(HEADS):
        c0, c1 = h * HD, (h + 1) * HD
        pieces = []
        c = c0
        while c < c1:
            mc = c // 128
            p0 = c - mc * 128
            p1 = min(128, c1 - mc * 128)
            pieces.append((mc, p0, p1, c - c0))
            c = mc * 128 + p1
        out.append(pieces)
    return out


def build(cfg, timing_reps=0):
    """cfg: dict(H, W, NB, k_diag, v_diag).

    timing_reps > 0: big I/O becomes internal DRAM (tiny token in/out only)
    and the whole body runs under For_i(timing_reps) for device-side timing.
    """
    H, W, NB = cfg["H"], cfg["W"], cfg["NB"]
    HW = H * W
    assert HW % NB == 0 and NB % 128 == 0 and NB % W == 0
    NBLK = HW // NB
    E = NB // 128          # 128-n chunks per block
    RB = NB // W           # rows per block
    EXT = NB + 2 * W       # ext columns (with halo)
    GOFF = 2               # left zero-guard cols (even => 4B-aligned dx=0 taps)
    GEXT = EXT + 2 * GOFF  # + zero guard cols each side
    # per-oc dwconv mode: 's' = all-DVE stt, 'd' = all-PE diag, 'h' = hybrid
    # (6 dx-taps on PE diag-matmul, 3 dy-taps + psum-combine on DVE)
    dw_modes = cfg.get("dw_modes") or ["h", "h", "h", "d", "d", "d"]
    assert len(dw_modes) == OC
    ablate = cfg.get("ablate", "full")   # conv | dw | kt | s | nop2 | full
    kt_mode = cfg.get("kt_mode", "inline")  # inline | spill
    v_stage = cfg.get("v_stage", False)
    timing = timing_reps > 0

    nc = bacc.Bacc("TRN2", target_bir_lowering=False)

    # ---- DRAM parameters ----
    if timing:
        tok_d = nc.declare_dram_parameter("tok", [1, 1], f32, isOutput=False)
        toko_d = nc.declare_dram_parameter("tok_out", [1, 1], f32, isOutput=True)
        x_d = nc.dram_tensor("x", [C, HW], bf16)
        qT_d = nc.dram_tensor("qT", [HW, C], bf16)
        y_d = nc.dram_tensor("y", [C, HW], f32)
    else:
        x_d = nc.declare_dram_parameter("x", [C, HW], bf16, isOutput=False)
        qT_d = nc.declare_dram_parameter("qT", [HW, C], bf16, isOutput=False)
        y_d = nc.declare_dram_parameter("y", [C, HW], f32, isOutput=True)
    wkv_d = nc.declare_dram_parameter("wkv", [128, CC, C2], bf16, isOutput=False)
    dws_d = nc.declare_dram_parameter("dws", [128, OC, 9], f32, isOutput=False)
    dwsn_d = nc.declare_dram_parameter("dwsn", [128, OC, 9], f32, isOutput=False)
    kvb_d = nc.declare_dram_parameter("kvb", [128, OC], f32, isOutput=False)
    dwb_d = nc.declare_dram_parameter("dwb", [128, OC], f32, isOutput=False)
    diag_ocs = [oc for oc in range(OC) if dw_modes[oc] in ("d", "h")]
    # slot layout: per diag oc, 'd' stores taps 0..8; 'h' stores the 6 dx!=0 taps
    diag_slots = {}
    nslot = 0
    for oc in diag_ocs:
        taps = list(range(9)) if dw_modes[oc] == "d" else [0, 2, 3, 5, 6, 8]
        diag_slots[oc] = {t: nslot + i for i, t in enumerate(taps)}
        nslot += len(taps)
    ndiag = len(diag_ocs)
    if ndiag:
        diag_d = nc.declare_dram_parameter("diag", [128, nslot, 128], bf16,
                                           isOutput=False)
    projT_d = nc.declare_dram_parameter("projT", [HD, HEADS, C], bf16, isOutput=False)
    projb_d = nc.declare_dram_parameter("projb", [128, CC], f32, isOutput=False)
    tempP_d = nc.declare_dram_parameter("tempP", [128, CC], f32, isOutput=False)

    v_dram = nc.dram_tensor("v_spill", [C, HW], bf16)
    k_dram = nc.dram_tensor("k_spill", [C, HW], bf16)
    rn_dram = nc.dram_tensor("rn_row", [C], f32)

    xv = x_d[:, :].rearrange("(cc p) n -> p cc n", p=128)
    yv = y_d[:, :].rearrange("(cc p) n -> p cc n", p=128)
    vv = v_dram[:, :].rearrange("(cc p) n -> p cc n", p=128)
    kv_sp = k_dram[:, :].rearrange("(cc p) n -> p cc n", p=128)
    qv = qT_d[:, :].rearrange("(g p) c -> p g c", p=128)

    diag_idx = {oc: i for i, oc in enumerate(diag_ocs)}

    TAPS = [(dy, dx) for dy in (-1, 0, 1) for dx in (-1, 0, 1)]
    CENTER = 4
    pieces = head_pieces()
    # per-mc d-range actually needed in the S gram (block-diagonal by head)
    s_drange = []
    for mc in range(CC):
        h_lo = (mc * 128) // HD
        h_hi = (min((mc + 1) * 128, C) - 1) // HD
        s_drange.append((h_lo * HD, (h_hi + 1) * HD))

    with tile.TileContext(nc) as tc, ExitStack() as ctx:
        const = ctx.enter_context(tc.tile_pool(name="const", bufs=1))
        wkv = const.tile([128, CC, C2], bf16)
        nc.sync.dma_start(out=wkv, in_=wkv_d[:, :, :])
        dws = const.tile([128, OC, 9], f32)
        nc.sync.dma_start(out=dws, in_=dws_d[:, :, :])
        dwsn = const.tile([128, OC, 9], f32)
        nc.sync.dma_start(out=dwsn, in_=dwsn_d[:, :, :])
        kvb = const.tile([128, OC], f32)
        nc.sync.dma_start(out=kvb, in_=kvb_d[:, :])
        dwb = const.tile([128, OC], f32)
        nc.sync.dma_start(out=dwb, in_=dwb_d[:, :])
        if ndiag:
            diag = const.tile([128, nslot, 128], bf16)
            nc.sync.dma_start(out=diag, in_=diag_d[:, :, :])
        projT = const.tile([HD, HEADS, C], bf16)
        nc.sync.dma_start(out=projT, in_=projT_d[:, :, :])
        projb = const.tile([128, CC], f32)
        nc.sync.dma_start(out=projb, in_=projb_d[:, :])
        tempP = const.tile([128, CC], f32)
        nc.sync.dma_start(out=tempP, in_=tempP_d[:, :])

        normacc = const.tile([128, CC * NBLK * (NB // PS)], f32)
        S_acc = const.tile([128, CC, C], f32)

        if timing:
            tokt = const.tile([1, 1], f32)
            nc.sync.dma_start(out=tokt, in_=tok_d[:, :])
            nc.sync.dma_start(out=toko_d[:, :], in_=tokt)
            loop_cm = tc.For_i(0, timing_reps, 1)
        else:
            loop_cm = contextlib.nullcontext()

        with loop_cm, ExitStack() as lctx:
            p1 = lctx.enter_context(ExitStack())
            xext = p1.enter_context(tc.tile_pool(name="xext", bufs=2))
            kv0p = p1.enter_context(tc.tile_pool(name="kv0", bufs=2))
            kblkp = p1.enter_context(tc.tile_pool(name="kblk", bufs=2))
            ktp = p1.enter_context(tc.tile_pool(name="kt", bufs=2))
            qtp = p1.enter_context(tc.tile_pool(name="qt", bufs=2))
            vstg = p1.enter_context(tc.tile_pool(name="vstg", bufs=4))
            need_vblk = (not v_stage) or any(m != "d" for m in dw_modes[CC:])
            if any(m == "c" for m in dw_modes):
                scrp2 = p1.enter_context(tc.tile_pool(name="scr2", bufs=2))
            if need_vblk:
                vblkp = p1.enter_context(tc.tile_pool(name="vblk", bufs=2))
            psc = p1.enter_context(tc.tile_pool(name="psc", bufs=2, space="PSUM"))
            sqp = p1.enter_context(tc.tile_pool(name="sqp", bufs=1, space="PSUM"))
            psd = p1.enter_context(tc.tile_pool(name="psd", bufs=2, space="PSUM"))
            pss = p1.enter_context(tc.tile_pool(name="pss", bufs=1, space="PSUM"))
            psS = pss.tile([128, CC, PS], f32)

            for blk in range(NBLK):
                n0 = blk * NB
                lo = max(0, n0 - W)
                hi = min(HW, n0 + NB + W)
                off = lo - (n0 - W)     # valid-region offset in ext tile

                xe = xext.tile([128, CC, EXT], bf16)
                if off > 0:
                    nc.vector.memset(xe[:, :, :off], 0.0)
                if (n0 - W) + EXT > hi:
                    nc.vector.memset(xe[:, :, off + (hi - lo):], 0.0)
                nc.gpsimd.dma_start(out=xe[:, :, off:off + (hi - lo)],
                                     in_=xv[:, :, lo:hi])

                kv0 = kv0p.tile([128, OC, GEXT], bf16)
                nchunks = [(i * PS, min(PS, EXT - i * PS))
                           for i in range((EXT + PS - 1) // PS)]
                for oc in range(OC):
                    for ci, (cs, cl) in enumerate(nchunks):
                        ps = psc.tile([128, PS], f32)
                        for cc in range(CC):
                            nc.tensor.matmul(
                                ps[:, :cl],
                                lhsT=wkv[:, cc, oc * 128:(oc + 1) * 128],
                                rhs=xe[:, cc, cs:cs + cl],
                                start=(cc == 0), stop=(cc == CC - 1))
                        dst = kv0[:, oc, GOFF + cs:GOFF + cs + cl]
                        if (oc + ci) % 3 == 0:
                            nc.vector.tensor_scalar_add(dst, ps[:, :cl],
                                                        kvb[:, oc:oc + 1])
                        else:
                            nc.scalar.activation(dst, ps[:, :cl], AF.Identity,
                                                 bias=kvb[:, oc:oc + 1])
                # zero image-boundary halos (incl. bias!) + guard cols
                if lo == 0:
                    nc.vector.memset(kv0[:, :, :GOFF + W], 0.0)
                else:
                    nc.vector.memset(kv0[:, :, 0:GOFF], 0.0)
                if hi == HW:
                    nc.vector.memset(kv0[:, :, GOFF + W + NB:], 0.0)
                else:
                    nc.vector.memset(kv0[:, :, GEXT - GOFF:], 0.0)

                k_blk = kblkp.tile([128, CC, NB], bf16)
                v_blk = vblkp.tile([128, CC, NB], bf16, tag="vblk", name="v_blk") if need_vblk else None

                def dw_stt(oc, dst):
                    """9-tap dwconv via DVE scalar_tensor_tensor chain."""
                    src3 = kv0[:, oc, GOFF:GOFF + EXT].rearrange("p (r w) -> p r w", w=W)
                    dst3 = dst.rearrange("p (r w) -> p r w", w=W)
                    nc.vector.tensor_scalar(
                        dst, kv0[:, oc, GOFF + W:GOFF + W + NB],
                        dws[:, oc, CENTER:CENTER + 1], dwb[:, oc:oc + 1],
                        ALU.mult, ALU.add)
                    for t, (dy, dx) in enumerate(TAPS):
                        if t == CENTER:
                            continue
                        sc = dws[:, oc, t:t + 1]
                        if dx == 0:
                            src = kv0[:, oc, GOFF + (1 + dy) * W:GOFF + (1 + dy) * W + NB]
                            nc.vector.scalar_tensor_tensor(
                                dst, src, sc, dst, ALU.mult, ALU.add)
                        elif dx == 1:
                            o3 = dst3[:, :, 0:W - 1]
                            s3 = src3[:, 1 + dy:1 + dy + RB, 1:W]
                            nc.vector.scalar_tensor_tensor(
                                o3, s3, sc, o3, ALU.mult, ALU.add)
                        else:
                            o3 = dst3[:, :, 1:W]
                            s3 = src3[:, 1 + dy:1 + dy + RB, 0:W - 1]
                            nc.vector.scalar_tensor_tensor(
                                o3, s3, sc, o3, ALU.mult, ALU.add)

                def dw_diag(oc, dst=None, spill_to=None):
                    """9-tap dwconv via PE diag-matmuls (flat shifted reads,
                    then subtract row-wrap contamination at w=0 / w=W-1).

                    dst: full [128, NB] destination (k chunks);
                    spill_to: (vv AP base col) - evict via small staging tiles
                    straight to DRAM (v chunks)."""
                    slots = diag_slots[oc]
                    rpc = PS // W
                    for nch in range(NB // PS):
                        ps = psd.tile([128, PS], f32, tag="psd")
                        r0 = nch * rpc + 1
                        for t, (dy, dx) in enumerate(TAPS):
                            lhsT = diag[:, slots[t], :]
                            base = GOFF + (r0 + dy) * W + dx
                            nc.tensor.matmul(ps, lhsT=lhsT,
                                             rhs=kv0[:, oc, base:base + PS],
                                             start=(t == 0), stop=(t == 8))
                        if dst is not None:
                            dsl = dst[:, nch * PS:(nch + 1) * PS]
                        else:
                            dsl = vstg.tile([128, PS], bf16, tag="vs")
                        if nch % 2 == 0:
                            nc.vector.tensor_scalar_add(dsl, ps, dwb[:, oc:oc + 1])
                        else:
                            nc.scalar.activation(dsl, ps, AF.Identity,
                                                 bias=dwb[:, oc:oc + 1])
                        if dst is None:
                            # wrap fixups on the staging chunk, then spill
                            for t, (dy, dx) in enumerate(TAPS):
                                if dx == 0:
                                    continue
                                sc = dwsn[:, oc, t:t + 1]
                                rr = nch * rpc  # first output row of chunk
                                if dx == 1:
                                    srcc = kv0[:, oc, GOFF + (rr + dy + 2) * W::W][:, :rpc]
                                    dstc = dsl[:, W - 1::W]
                                else:
                                    srcc = kv0[:, oc, GOFF - 1 + (rr + dy + 1) * W::W][:, :rpc]
                                    dstc = dsl[:, 0::W]
                                nc.vector.scalar_tensor_tensor(
                                    dstc, srcc, sc, dstc, ALU.mult, ALU.add)
                            nc.gpsimd.dma_start(
                                out=spill_to[:, nch * PS:(nch + 1) * PS], in_=dsl)
                    if dst is not None:
                        for t, (dy, dx) in enumerate(TAPS):
                            if dx == 0:
                                continue
                            sc = dwsn[:, oc, t:t + 1]
                            if dx == 1:
                                srcc = kv0[:, oc, GOFF + (dy + 2) * W::W][:, :RB]
                                dstc = dst[:, W - 1::W]
                            else:
                                srcc = kv0[:, oc, GOFF - 1 + (dy + 1) * W::W][:, :RB]
                                dstc = dst[:, 0::W]
                            nc.vector.scalar_tensor_tensor(
                                dstc, srcc, sc, dstc, ALU.mult, ALU.add)

                def dw_copy(oc, dst):
                    """All-DVE dwconv with a +1-shifted kv0 copy so every
                    tap read is 4B-aligned (2x mode); center tap runs 4x."""
                    kvm = scrp2.tile([128, GEXT], bf16, tag="kvm")
                    nc.vector.tensor_copy(kvm[:, 0:GEXT - 2],
                                          kv0[:, oc, 1:GEXT - 1])
                    # kvm[j] = kv0[j+1]
                    nc.vector.tensor_scalar(
                        dst, kv0[:, oc, GOFF + W:GOFF + W + NB],
                        dws[:, oc, CENTER:CENTER + 1], dwb[:, oc:oc + 1],
                        ALU.mult, ALU.add)
                    for t, (dy, dx) in enumerate(TAPS):
                        if t == CENTER:
                            continue
                        sc = dws[:, oc, t:t + 1]
                        if dx == 0:
                            srcf = kv0[:, oc, GOFF + (1 + dy) * W:
                                       GOFF + (1 + dy) * W + NB]
                        elif dx == 1:
                            srcf = kvm[:, GOFF + (1 + dy) * W - 1 + 1:
                                       GOFF + (1 + dy) * W + NB]
                        else:
                            srcf = kvm[:, GOFF + (1 + dy) * W - 2:
                                       GOFF + (1 + dy) * W - 2 + NB]
                        nc.vector.scalar_tensor_tensor(
                            dst, srcf, sc, dst, ALU.mult, ALU.add)
                    # wrap fixups (subtract contamination at w edges)
                    for t, (dy, dx) in enumerate(TAPS):
                        if dx == 0:
                            continue
                        sc = dwsn[:, oc, t:t + 1]
                        if dx == 1:
                            srcc = kv0[:, oc, GOFF + (dy + 2) * W::W][:, :RB]
                            dstc = dst[:, W - 1::W]
                        else:
                            srcc = kv0[:, oc, GOFF - 1 + (dy + 1) * W::W][:, :RB]
                            dstc = dst[:, 0::W]
                        nc.vector.scalar_tensor_tensor(
                            dstc, srcc, sc, dstc, ALU.mult, ALU.add)

                def dw_hyb(oc, dst):
                    """dx!=0 taps (6) on PE diag-matmuls; center (4x) + one dy
                    tap (2x) on DVE; last dy tap fused with psum combine."""
                    slots = diag_slots[oc]
                    rpc = PS // W
                    # center tap + bias (TS, 4x)
                    nc.vector.tensor_scalar(
                        dst, kv0[:, oc, GOFF + W:GOFF + W + NB],
                        dws[:, oc, CENTER:CENTER + 1], dwb[:, oc:oc + 1],
                        ALU.mult, ALU.add)
                    # dy=-1 tap (2x)
                    nc.vector.scalar_tensor_tensor(
                        dst, kv0[:, oc, GOFF:GOFF + NB],
                        dws[:, oc, 1:2], dst, ALU.mult, ALU.add)
                    for nch in range(NB // PS):
                        ps = psd.tile([128, PS], f32, tag="psd")
                        r0 = nch * rpc + 1
                        first = True
                        for t, (dy, dx) in enumerate(TAPS):
                            if dx == 0:
                                continue
                            base = GOFF + (r0 + dy) * W + dx
                            nc.tensor.matmul(ps, lhsT=diag[:, slots[t], :],
                                             rhs=kv0[:, oc, base:base + PS],
                                             start=first, stop=(t == 8))
                            first = False
                        # dy=+1 tap, then add the PE psum partial
                        dsl = dst[:, nch * PS:(nch + 1) * PS]
                        base = GOFF + (2 + nch * rpc) * W
                        nc.vector.scalar_tensor_tensor(
                            dsl, kv0[:, oc, base:base + PS],
                            dws[:, oc, 7:8], dsl, ALU.mult, ALU.add)
                        nc.vector.tensor_tensor(dsl, dsl, ps, ALU.add)
                    # wrap fixups for the 6 PE dx-taps
                    for t, (dy, dx) in enumerate(TAPS):
                        if dx == 0:
                            continue
                        sc = dwsn[:, oc, t:t + 1]
                        if dx == 1:
                            srcc = kv0[:, oc, GOFF + (dy + 2) * W::W][:, :RB]
                            dstc = dst[:, W - 1::W]
                        else:
                            srcc = kv0[:, oc, GOFF - 1 + (dy + 1) * W::W][:, :RB]
                            dstc = dst[:, 0::W]
                        nc.vector.scalar_tensor_tensor(
                            dstc, srcc, sc, dstc, ALU.mult, ALU.add)

                if ablate != "conv":
                    for oc in range(OC):
                        m = dw_modes[oc]
                        if oc >= CC and m == "d" and v_stage:
                            dw_diag(oc, spill_to=vv[:, oc - CC, n0:n0 + NB])
                            continue
                        dst = k_blk[:, oc, :] if oc < CC else v_blk[:, oc - CC, :]
                        if m == "d":
                            dw_diag(oc, dst)
                        elif m == "h":
                            dw_hyb(oc, dst)
                        elif m == "c":
                            dw_copy(oc, dst)
                        else:
                            dw_stt(oc, dst)
                    if need_vblk and ablate in ("full", "s", "kt", "dw"):
                        for vc in range(CC):
                            if dw_modes[CC + vc] != "d" or not v_stage:
                                nc.gpsimd.dma_start(
                                    out=vv[:, vc, n0:n0 + NB],
                                    in_=v_blk[:, vc, :])

                if ablate in ("full", "s", "kt", "dw"):
                    nsub = NB // PS
                    for cc in range(CC):
                        for j in range(nsub):
                            sq = sqp.tile([128, PS], f32, tag="sq")
                            slot = (blk * CC + cc) * nsub + j
                            nc.scalar.activation(
                                sq, k_blk[:, cc, j * PS:(j + 1) * PS], AF.Square,
                                accum_out=normacc[:, slot:slot + 1])

                if ablate in ("full", "s", "kt"):
                    kT = ktp.tile([128, E, CC * 128], bf16)
                    if kt_mode == "spill":
                        nc.gpsimd.dma_start(out=kv_sp[:, :, n0:n0 + NB], in_=k_blk)
                        nc.sync.dma_start_transpose(
                            out=kT, in_=k_dram[:, n0:n0 + NB])
                    else:
                        for cc in range(CC):
                            nc.sync.dma_start_transpose(
                                out=kT[:, :, cc * 128:(cc + 1) * 128],
                                in_=k_blk[:, cc, :])

                if ablate in ("full", "s"):
                    qt = qtp.tile([128, E, C], bf16, tag="qt")
                    nc.gpsimd.dma_start(out=qt, in_=qv[:, blk * E:(blk + 1) * E, :])
                    for e in range(E):
                        for mc in range(CC):
                            d0, d1 = s_drange[mc]
                            nc.tensor.matmul(
                                psS[:, mc, d0:d1],
                                lhsT=qt[:, e, mc * 128:(mc + 1) * 128],
                                rhs=kT[:, e, d0:d1],
                                start=(blk == 0 and e == 0),
                                stop=(blk == NBLK - 1 and e == E - 1))


            # ---------- middle ----------
            if ablate == "full":
                for mc in range(CC):
                    d0, d1 = s_drange[mc]
                    nc.vector.tensor_copy(S_acc[:, mc, d0:d1], psS[:, mc, d0:d1])
            p1.close()
            if ablate == "full":
                mid = lctx.enter_context(tc.tile_pool(name="mid", bufs=1))
                psm = lctx.enter_context(tc.tile_pool(name="psm", bufs=2, space="PSUM"))
                n2 = mid.tile([128, CC], f32)
                nc.vector.tensor_reduce(
                    n2, normacc.rearrange("p (b c j) -> p c b j", c=CC,
                                          j=NB // PS),
                    axis=AX.XY, op=ALU.add)
                sn = mid.tile([128, CC], f32)
                nc.scalar.activation(sn, n2, AF.Sqrt)
                nc.vector.tensor_scalar_max(sn, sn, 1e-12)
                rn = mid.tile([128, CC], f32)
                nc.vector.reciprocal(rn, sn)
                t1 = mid.tile([128, CC], f32)
                nc.vector.tensor_tensor(t1, rn, rn, ALU.mult)
                nc.vector.tensor_tensor(t1, t1, n2, ALU.mult)
                nc.vector.tensor_scalar(t1, t1, -0.5, 1.5, ALU.mult, ALU.add)
                nc.vector.tensor_tensor(rn, rn, t1, ALU.mult)
                nc.vector.tensor_tensor(rn, rn, tempP, ALU.mult)
                nc.sync.dma_start(out=rn_dram[:].rearrange("(c p) -> p c", p=128),
                                  in_=rn)
                rnb = mid.tile([HD, C], f32)
                rn_bcast_src = bass.AP(tensor=rn_dram, offset=0, ap=[[0, HD], [1, C]])
                nc.gpsimd.dma_start(out=rnb, in_=rn_bcast_src)

                lg = mid.tile([HD, HEADS, HD], f32)
                for h, pl in enumerate(pieces):
                    for (mc, p0, p1_, s0) in pl:
                        nc.sync.dma_start(
                            out=lg[s0:s0 + (p1_ - p0), h, :],
                            in_=S_acc[p0:p1_, mc, h * HD:(h + 1) * HD])
                lg2 = lg.rearrange("p h d -> p (h d)")
                nc.vector.tensor_tensor(lg2, lg2, rnb, ALU.mult)
                mx = mid.tile([HD, HEADS], f32)
                nc.vector.tensor_reduce(mx, lg, axis=AX.X, op=ALU.max)
                nc.vector.tensor_tensor(
                    lg, lg, mx[:, :, None].broadcast_to([HD, HEADS, HD]), ALU.subtract)
                nc.scalar.activation(lg2, lg2, AF.Exp)
                sm = mid.tile([HD, HEADS], f32)
                nc.vector.tensor_reduce(sm, lg, axis=AX.X, op=ALU.add)
                nc.vector.reciprocal(sm, sm)
                nc.vector.tensor_tensor(
                    lg, lg, sm[:, :, None].broadcast_to([HD, HEADS, HD]), ALU.mult)
                attnb = mid.tile([HD, HEADS, HD], bf16)
                nc.vector.tensor_copy(attnb, lg)

                MbT = mid.tile([128, CC, C], bf16)
                for h in range(HEADS):
                    pm = psm.tile([HD, C], f32)
                    nc.tensor.matmul(pm, lhsT=attnb[:, h, :], rhs=projT[:, h, :],
                                     start=True, stop=True)
                    msc = mid.tile([HD, C], bf16, tag="msc")
                    nc.vector.tensor_copy(msc, pm)
                    for (mc, p0, p1_, s0) in pieces[h]:
                        nc.sync.dma_start(out=MbT[p0:p1_, mc, :],
                                          in_=msc[s0:s0 + (p1_ - p0), :])

                # ---------- pass 2 ----------
                p2 = lctx.enter_context(ExitStack())
                vbp = p2.enter_context(tc.tile_pool(name="vb2", bufs=2))
                outp = p2.enter_context(tc.tile_pool(name="outp", bufs=3))
                psf = p2.enter_context(tc.tile_pool(name="psf", bufs=2, space="PSUM"))
                for blk in range(NBLK):
                    n0 = blk * NB
                    vb = vbp.tile([128, CC, NB], bf16)
                    nc.gpsimd.dma_start(out=vb, in_=vv[:, :, n0:n0 + NB])
                    for oc in range(CC):
                        ot = outp.tile([128, NB], f32)
                        for nch in range(NB // PS):
                            ps = psf.tile([128, PS], f32)
                            for dc in range(CC):
                                nc.tensor.matmul(
                                    ps,
                                    lhsT=MbT[:, dc, oc * 128:(oc + 1) * 128],
                                    rhs=vb[:, dc, nch * PS:(nch + 1) * PS],
                                    start=(dc == 0), stop=(dc == CC - 1))
                            dsl = ot[:, nch * PS:(nch + 1) * PS]
                            if nch % 2 == 0:
                                nc.scalar.activation(dsl, ps, AF.Identity,
                                                     bias=projb[:, oc:oc + 1])
                            else:
                                nc.vector.tensor_scalar_add(dsl, ps, projb[:, oc:oc + 1])
                        nc.gpsimd.dma_start(out=yv[:, oc, n0:n0 + NB], in_=ot)
                p2.close()

    nc.compile()
    return nc


def host_prep(inputs, cfg):
    """Full inputs (numpy, reference layout) -> per-core in_maps list."""
    H, W = cfg["H"], cfg["W"]
    HW = H * W
    x = np.ascontiguousarray(inputs["x"]).reshape(-1, C, HW)
    B = x.shape[0]
    qp = np.asarray(inputs["q_param"])[0]              # [heads, hd, 48]
    temp = np.asarray(inputs["temperature"]).reshape(HEADS)
    kv_w = np.asarray(inputs["kv_w"])[:, :, 0, 0]      # [768, 384]
    kv_b = np.asarray(inputs["kv_b"])
    dw_w = np.asarray(inputs["dw_w"])[:, 0]            # [768, 3, 3]
    dw_b = np.asarray(inputs["dw_b"])
    pw = np.asarray(inputs["proj_w"])[:, :, 0, 0]      # [384, 384]
    pb = np.asarray(inputs["proj_b"])

    idx = (np.arange(HW) * qp.shape[-1]) // HW
    q = qp[:, :, idx].reshape(C, HW)
    qT = np.ascontiguousarray(q.T).astype(ml_dtypes.bfloat16)

    wkv = np.ascontiguousarray(
        kv_w.T.reshape(CC, 128, C2).transpose(1, 0, 2)).astype(ml_dtypes.bfloat16)
    dws = np.ascontiguousarray(
        dw_w.reshape(OC, 128, 9).transpose(1, 0, 2)).astype(np.float32)
    kvb = np.ascontiguousarray(kv_b.reshape(OC, 128).T).astype(np.float32)
    dwb = np.ascontiguousarray(dw_b.reshape(OC, 128).T).astype(np.float32)

    dw_modes = cfg.get("dw_modes") or ["h", "h", "h", "d", "d", "d"]
    diag_ocs = [oc for oc in range(6) if dw_modes[oc] in ("d", "h")]
    slot_list = []
    for oc in diag_ocs:
        taps = list(range(9)) if dw_modes[oc] == "d" else [0, 2, 3, 5, 6, 8]
        for t in taps:
            slot_list.append((oc, t))
    ndiag = len(diag_ocs)
    diag = np.zeros((128, max(len(slot_list), 1), 128), np.float32)
    for i, (oc, t) in enumerate(slot_list):
        dy, dx = t // 3 - 1, t % 3 - 1
        s = dw_w[oc * 128:(oc + 1) * 128, dy + 1, dx + 1]
        diag[np.arange(128), i, np.arange(128)] = s
    diag = diag.astype(ml_dtypes.bfloat16)

    projT = np.ascontiguousarray(
        pw.T.reshape(HEADS, HD, C).transpose(1, 0, 2)).astype(ml_dtypes.bfloat16)
    projb = np.ascontiguousarray(pb.reshape(CC, 128).T).astype(np.float32)
    tempP = np.zeros((128, CC), np.float32)
    for cc in range(CC):
        for p in range(128):
            tempP[p, cc] = temp[(cc * 128 + p) // HD]

    shared = dict(qT=qT, wkv=wkv, dws=dws, dwsn=(-dws).astype(np.float32),
                  kvb=kvb, dwb=dwb, projT=projT, projb=projb, tempP=tempP)
    if ndiag:
        shared["diag"] = diag
    in_maps = []
    for b in range(B):
        m = dict(shared)
        m["x"] = x[b].astype(ml_dtypes.bfloat16)
        in_maps.append(m)
    return in_maps


# ---------------------------------------------------------------------------
# Harness entry point: kernel(**inputs) -> full output (B, C, H, W) float32.
#
# Shards batch across the 8 NeuronCores (data parallel, no collectives),
# runs the Bass program via run_bass_kernel_spmd, gathers per-core outputs.
# ---------------------------------------------------------------------------

CFG = dict(H=128, W=128, NB=2048, dw_modes=["h", "h", "h", "h", "d", "d"])

_PROGRAM_CACHE = {}


def _get_program():
    key = "main"
    if key not in _PROGRAM_CACHE:
        _PROGRAM_CACHE[key] = build(CFG)
    return _PROGRAM_CACHE[key]


def kernel(**inputs):
    from concourse.bass_utils import run_bass_kernel_spmd

    x = np.asarray(inputs["x"])
    B, Cin, H, W_ = x.shape
    assert (Cin, H, W_) == (C, CFG["H"], CFG["W"]) and B == 8
    nc = _get_program()
    in_maps = host_prep(inputs, CFG)
    res = run_bass_kernel_spmd(nc, in_maps, list(range(8)))
    out = np.stack([res.results[b]["y"].reshape(C, H, W_) for b in range(B)])
    return out.astype(np.float32)

